# revision 1
# baseline (speedup 1.0000x reference)
"""CosFace loss kernel for Trainium2, sharded across 8 NeuronCores.

Problem: x[512,512], gt[512] (int64), wt[100000,512] ->
    mean_b( -log_softmax(64*(cos(x_b, wt) - 0.35*onehot(gt_b)))[gt_b] )

Strategy (vocab/tensor parallel, per the standard sharded large-vocab CE):
  - Host: L2-normalize wt rows and x rows (f32), transpose to [D, C] layout
    (so the contraction dim D lands on SBUF partitions), cast bf16, shard wt
    along the class dim into 8 x [512, 12500].
  - Device (per core): z = xn^T wtn via PE (bf16, f32 accum),
    S_k[b] = sum_c exp(64*z[b,c] - 64) via fused ACT exp+row-sum.
    The shift 64 is a provable bound on the logits (|cos| <= 1), so no
    running max / cross-shard max is needed.
  - Host: all-reduce S_k by summing (8x512 floats), gather the target
    logit t_b = cos(x_b, wt[gt_b]) (one of the shards owns it; equivalent
    host-side gather), patch the margin into S, and finish:
       nll_b = (64 + log(S_b - e^{64 t_b - 64} + e^{64(t_b - .35) - 64}))
               - 64*(t_b - 0.35)
"""

import numpy as np
import ml_dtypes

N_CORES = 8
B, D, C = 512, 512, 100000
CS = C // N_CORES  # 12500 classes per core
P = 128
KT = D // P  # 4 contraction tiles
BT = B // P  # 4 output-row tiles
CHUNK = 500  # matmul free dim / psum bank (500 f32 <= 512)
NCHUNK = CS // CHUNK  # 25
SC = 2500  # DMA super-chunk (columns per wt DMA)
NSC = CS // SC  # 5
SCALE = 64.0
MARGIN = 0.35
EPS = 1e-12
SHIFT = 64.0

# fp8 variant parameters
F8_CHUNK = 512
F8_CS_PAD = 12800  # 25 chunks of 512; 300 zero-pad classes per core
F8_NCHUNK = F8_CS_PAD // F8_CHUNK  # 25
F8_PRESCALE = 32.0  # cast (32*xn), (32*wn) to fp8; z' = 1024*z
F8_EXP_SCALE = SCALE / (F8_PRESCALE * F8_PRESCALE)  # 1/16, exact
F8_PAD_TERMS = (F8_CS_PAD - CS) * N_CORES  # zero-pad exp(+0-64) terms

_CACHE = {}


def _build_bass(reps=1):
    """Build the per-core device module.

    reps>1 repeats the full pipeline (including DMAs) back to back; used
    only for wall-clock delta timing ((t_R - t_1)/(R-1) cancels the axon
    dispatch overhead). The graded path uses reps=1.
    """
    import concourse.mybir as mybir
    import concourse.tile as tile
    from concourse import bacc

    nc = bacc.Bacc("TRN2", target_bir_lowering=False, debug=False,
                   num_devices=N_CORES)
    xt = nc.dram_tensor("xt", [D, B], mybir.dt.bfloat16,
                        kind="ExternalInput").ap()
    wtt = nc.dram_tensor("wtt", [D, CS], mybir.dt.bfloat16,
                         kind="ExternalInput").ap()
    s_out = nc.dram_tensor("s_out", [BT, P], mybir.dt.float32,
                           kind="ExternalOutput").ap()

    with tile.TileContext(nc) as tc:
        with (
            tc.tile_pool(name="xp", bufs=1) as xp,
            tc.tile_pool(name="wp", bufs=1) as wp,
            tc.tile_pool(name="accp", bufs=1) as accp,
            tc.tile_pool(name="scrp", bufs=4) as scrp,
            tc.tile_pool(name="outp", bufs=2) as outp,
            tc.tile_pool(name="psp", bufs=8, space="PSUM") as psp,
        ):
            # bias constant for the fused exp: exp(SCALE*z - SHIFT)
            bias_t = xp.tile([P, 1], mybir.dt.float32, name="bias_c",
                             tag="bias_c")
            nc.any.memset(bias_t[:], -SHIFT)

            for rep in range(reps):
                # x^T (normalized, bf16): 4 tiles [128d, 512b], resident
                xtiles = []
                for k in range(KT):
                    xk = xp.tile([P, B], mybir.dt.bfloat16, name=f"x{k}",
                                 tag=f"x{k}")
                    nc.sync.dma_start(xk[:], xt[k * P:(k + 1) * P, :])
                    xtiles.append(xk)

                # wt^T (normalized, bf16) shard: 4 x 5 tiles [128d, 2500c],
                # all resident (4*5*5000B = 100KB/partition)
                wtiles = {}
                for sc in range(NSC):
                    for k in range(KT):
                        wk = wp.tile([P, SC], mybir.dt.bfloat16,
                                     name=f"w{k}_{sc}", tag=f"w{k}_{sc}")
                        nc.sync.dma_start(
                            wk[:],
                            wtt[k * P:(k + 1) * P, sc * SC:(sc + 1) * SC])
                        wtiles[(k, sc)] = wk

                # per-row partials: acc[bt][:, ch] = sum_c exp(...) of chunk
                accs = []
                for bt in range(BT):
                    a = accp.tile([P, NCHUNK], mybir.dt.float32,
                                  name=f"acc{bt}", tag=f"acc{bt}")
                    accs.append(a)

                for ch in range(NCHUNK):
                    sc, off = divmod(ch * CHUNK, SC)
                    for bt in range(BT):
                        ps = psp.tile([P, CHUNK], mybir.dt.float32, name="ps",
                                      tag="ps")
                        for k in range(KT):
                            nc.tensor.matmul(
                                ps[:],
                                xtiles[k][:, bt * P:(bt + 1) * P],
                                wtiles[(k, sc)][:, off:off + CHUNK],
                                start=(k == 0),
                                stop=(k == KT - 1),
                            )
                        scr = scrp.tile([P, CHUNK], mybir.dt.float32,
                                        name="scr", tag="scr")
                        nc.scalar.activation(
                            scr[:], ps[:], mybir.ActivationFunctionType.Exp,
                            bias=bias_t[:], scale=SCALE,
                            accum_out=accs[bt][:, ch:ch + 1])

                for bt in range(BT):
                    st = outp.tile([P, 1], mybir.dt.float32, name=f"s{bt}",
                                   tag="s")
                    nc.vector.tensor_reduce(
                        st[:], accs[bt][:], axis=mybir.AxisListType.X,
                        op=mybir.AluOpType.add)
                    nc.sync.dma_start(s_out[bt:bt + 1, :], st[:])

    nc.compile()
    return nc


def _build_bass_fp8(reps=1):
    """fp8e4m3 DoubleRow variant: half the PE time and HBM traffic of bf16.

    Layout: xt8 [128, 4, 512] (partition=d%128, plane=d//128, b),
    wtt8 [128, 4, 12800] likewise with 300 zero-pad classes. DoubleRow
    matmuls contract two 128-row k-planes at once: 2 MMs per (chunk, bt).
    """
    import concourse.mybir as mybir
    import concourse.tile as tile
    from concourse import bacc

    nc = bacc.Bacc("TRN2", target_bir_lowering=False, debug=False,
                   num_devices=N_CORES)
    f8 = mybir.dt.float8e4
    xt = nc.dram_tensor("xt8", [P, KT, B], f8, kind="ExternalInput").ap()
    wtt = nc.dram_tensor("wtt8", [P, KT, F8_CS_PAD], f8,
                         kind="ExternalInput").ap()
    s_out = nc.dram_tensor("s_out", [BT, P], mybir.dt.float32,
                           kind="ExternalOutput").ap()

    SCW = 2560  # wt tile columns (5 chunks)
    NSCW = F8_CS_PAD // SCW  # 5
    NPAIR = KT // 2  # 2 DoubleRow k-pair groups
    GRP = 4  # psum banks (chunks) per fused exp+reduce ACT instruction
    groups = [list(range(g, min(g + GRP, F8_NCHUNK)))
              for g in range(0, F8_NCHUNK, GRP)]  # 6x4 + 1x1

    def chunk_width(ch):
        # the real shard is CS=12500 classes; the last chunk only computes
        # the 212 live columns so the 300 zero-pad classes cost nothing
        return min(F8_CHUNK, CS - ch * F8_CHUNK)

    with tile.TileContext(nc) as tc:
        with (
            tc.tile_pool(name="xp", bufs=1) as xp,
            tc.tile_pool(name="wp", bufs=1) as wp,
            tc.tile_pool(name="accp", bufs=1) as accp,
            tc.tile_pool(name="scrp", bufs=4) as scrp,
            tc.tile_pool(name="outp", bufs=2) as outp,
            tc.tile_pool(name="psp", bufs=2, space="PSUM") as psp,
        ):
            bias_t = xp.tile([P, 1], mybir.dt.float32, name="bias_c",
                             tag="bias_c")
            nc.any.memset(bias_t[:], -SHIFT)
            # dummy exp to pull ACT_TABLE_LOAD (~2.7us) under the initial DMAs
            warm = xp.tile([P, 1], mybir.dt.float32, name="warm", tag="warm")
            nc.scalar.activation(warm[:], bias_t[:],
                                 mybir.ActivationFunctionType.Exp,
                                 bias=bias_t[:], scale=1.0)

            for rep in range(reps):
                xtile = xp.tile([P, KT, B], f8, name="x8", tag="x8")
                nc.sync.dma_start(xtile[:], xt[:])

                wtiles = {}
                for sc in range(NSCW):
                    for pr in range(NPAIR):
                        wk = wp.tile([P, 2, SCW], f8, name=f"w{pr}_{sc}",
                                     tag=f"w{pr}_{sc}")
                        src = wtt[:, 2 * pr:2 * pr + 2,
                                  sc * SCW:(sc + 1) * SCW]
                        if sc == 0:
                            # chunk-granular pieces so the first matmuls
                            # start after ~0.26MB instead of ~1.3MB
                            for j in range(SCW // F8_CHUNK):
                                sl = slice(j * F8_CHUNK, (j + 1) * F8_CHUNK)
                                nc.sync.dma_start(wk[:, :, sl], src[:, :, sl])
                        else:
                            nc.sync.dma_start(wk[:], src)
                        wtiles[(pr, sc)] = wk

                accs = []
                for bt in range(BT):
                    a = accp.tile([P, len(groups)], mybir.dt.float32,
                                  name=f"acc{bt}", tag=f"acc{bt}")
                    accs.append(a)

                for gi, chs in enumerate(groups):
                    for bt in range(BT):
                        # one 4-bank psum group per (group, bt)
                        ps = psp.tile([P, GRP, F8_CHUNK], mybir.dt.float32,
                                      name="ps", tag="ps")
                        for ci, ch in enumerate(chs):
                            sc, off = divmod(ch * F8_CHUNK, SCW)
                            w = chunk_width(ch)
                            for pr in range(NPAIR):
                                nc.tensor.matmul(
                                    ps[:, ci, :w],
                                    xtile[:, 2 * pr:2 * pr + 2,
                                          bt * P:(bt + 1) * P],
                                    wtiles[(pr, sc)][:, :, off:off + w],
                                    start=(pr == 0),
                                    stop=(pr == NPAIR - 1),
                                    perf_mode=mybir.MatmulPerfMode.DoubleRow,
                                )
                        scr = scrp.tile([P, GRP, F8_CHUNK], mybir.dt.float32,
                                        name="scr", tag="scr")
                        n = len(chs)
                        lastw = chunk_width(chs[-1])
                        if lastw == F8_CHUNK:
                            in_ap = ps[:, :n, :]
                            out_ap = scr[:, :n, :]
                        elif n == 1:
                            in_ap = ps[:, 0, :lastw]
                            out_ap = scr[:, 0, :lastw]
                        else:  # full chunks + narrow tail: two activates
                            raise NotImplementedError
                        nc.scalar.activation(
                            out_ap, in_ap,
                            mybir.ActivationFunctionType.Exp,
                            bias=bias_t[:], scale=F8_EXP_SCALE,
                            accum_out=accs[bt][:, gi:gi + 1])

                for bt in range(BT):
                    st = outp.tile([P, 1], mybir.dt.float32, name=f"s{bt}",
                                   tag="s")
                    nc.vector.tensor_reduce(
                        st[:], accs[bt][:], axis=mybir.AxisListType.X,
                        op=mybir.AluOpType.add)
                    nc.sync.dma_start(s_out[bt:bt + 1, :], st[:])

    nc.compile()
    return nc


def _host_prep_fp8(xn, wn):
    import concourse.mybir as mybir

    f8np = mybir.dt.np(mybir.dt.float8e4)
    xs = np.ascontiguousarray(xn.T) * np.float32(F8_PRESCALE)  # [D, B]
    xt8 = np.ascontiguousarray(
        xs.reshape(KT, P, B).transpose(1, 0, 2)).astype(f8np)

    shards = []
    for c in range(N_CORES):
        ws = np.ascontiguousarray(
            wn[c * CS:(c + 1) * CS].T) * np.float32(F8_PRESCALE)  # [D, CS]
        w3 = ws.reshape(KT, P, CS).transpose(1, 0, 2)  # [P, KT, CS]
        wp = np.zeros((P, KT, F8_CS_PAD), dtype=f8np)
        wp[:, :, :CS] = w3.astype(f8np)
        shards.append(wp)
    return xt8, shards


def _get_bass(reps=1, impl="bf16"):
    key = ("nc", impl, reps)
    if key not in _CACHE:
        builder = _build_bass_fp8 if impl == "fp8" else _build_bass
        _CACHE[key] = builder(reps)
    return _CACHE[key]


def _normalize(x, wt):
    x = np.asarray(x, dtype=np.float32)
    wt = np.asarray(wt, dtype=np.float32)
    xn = x / np.maximum(np.sqrt((x * x).sum(axis=1, keepdims=True)), EPS)
    w_norm = np.sqrt(np.einsum("cd,cd->c", wt, wt, dtype=np.float32))
    inv = (1.0 / np.maximum(w_norm, EPS)).astype(np.float32)
    wn = wt * inv[:, None]
    return xn, wn


def _host_prep(x, wt):
    bf16 = ml_dtypes.bfloat16
    xn, wn = _normalize(x, wt)
    xt_bf = np.ascontiguousarray(xn.T).astype(bf16)  # [D, B]
    wtt_shards = []
    for c in range(N_CORES):
        sh = wn[c * CS:(c + 1) * CS]  # [CS, D]
        wtt_shards.append(np.ascontiguousarray(sh.T).astype(bf16))  # [D, CS]
    return xn, wn, xt_bf, wtt_shards


IMPL = "fp8"  # "bf16" or "fp8"


def kernel(x, gt, wt):
    import os
    from concourse.bass_utils import run_bass_kernel_spmd

    impl = os.environ.get("KERNEL_IMPL", IMPL)
    gt = np.asarray(gt).astype(np.int64)

    reps = int(os.environ.get("KERNEL_REPS", "1"))
    nc = _get_bass(reps, impl)
    if impl == "fp8":
        xn, wn = _normalize(x, wt)
        xt8, w8_shards = _host_prep_fp8(xn, wn)
        in_maps = [{"xt8": xt8, "wtt8": w8_shards[c]} for c in range(N_CORES)]
    else:
        xn, wn, xt_bf, wtt_shards = _host_prep(x, wt)
        in_maps = [{"xt": xt_bf, "wtt": wtt_shards[c]}
                   for c in range(N_CORES)]
    trace = bool(os.environ.get("KERNEL_TRACE"))
    res = run_bass_kernel_spmd(nc, in_maps, core_ids=list(range(N_CORES)),
                               trace=trace)
    _CACHE["last_results"] = res

    # all-reduce of the per-shard sums (8 x 512 floats) on host
    S = np.zeros(B, dtype=np.float64)
    for r in res.results:
        S += r["s_out"].astype(np.float64).reshape(B)
    # target logit, computed from the same normalized f32 tensors
    t = np.einsum("bd,bd->b", xn, wn[gt], dtype=np.float64)  # cos(x_b, w_gt)

    # patch the margin into the partition function and finish the loss
    S_adj = S - np.exp(SCALE * t - SHIFT) + np.exp(SCALE * (t - MARGIN) - SHIFT)
    nll = (SHIFT + np.log(S_adj)) - SCALE * (t - MARGIN)
    return np.asarray(nll.mean(), dtype=np.float32)

